# revision 4
# baseline (speedup 1.0000x reference)
"""GreedySampler kernel for 8 Trainium2 NeuronCores.

Memory-bound problem: streaming the 823MB embedding matrix dominates.

  * Host: gather the 200 "last token" rows of hidden_states, transpose
    to the PE's [K, M] layout, cast to bf16. Transpose embd_weight to
    [d, vocab], pad vocab to 51200, cast to bf16, shard over vocab into
    8 slices of 6400 columns (tensor parallel over vocab).
  * Device (SPMD, 8 cores): stream the 52MB W-shard once. W chunks are
    the stationary matmul operand ([128 d, 128 vocab]), the 200 job
    rows are the moving operand, so the PE runs at its theoretical
    floor (200*4096*6400 MACs / 128^2 = 320k cycles/core), below the
    ~146us DMA floor. bf16 matmuls accumulate logits.T[vocab, jobs] in
    PSUM over 32 K-chunks; results go out as bf16.
  * Host: per-row max over the gathered approximate logits, take every
    column within DELTA of the max (bf16 logit error is ~3e-3 abs;
    DELTA=0.25 is a ~50 sigma margin) and rescore those few candidates
    exactly in float64 against the original fp32 weights. The argmax
    over exact scores equals the fp32 reference argmax.

bf16 + exact-rescore halves DMA traffic AND runs the PE at full rate
(fp32 matmul is 4 cycles/row), while returning exact argmax indices.

This walrus build rejects instructions carrying more than one sync
wait, so after Tile scheduling we split excess waits onto same-engine
nop instructions inserted just before the offender (same engine queue,
in-order, so the semantics are identical).
"""

import math

import numpy as np
import ml_dtypes

import concourse.bass as bass
import concourse.mybir as mybir
import concourse.tile as tile
from concourse.vector_clock import ScopedClock
from concourse.bass_utils import run_bass_kernel_spmd

P = 128
N_CORES = 8
VG_W = 512  # W-tile width in vocab (4 stationary tiles of 128)
DELTA = 0.25  # host rescore margin (bf16 logit abs error is ~3e-3)

BF16 = mybir.dt.bfloat16
F32 = mybir.dt.float32

_drain_patched = False


def _patch_tile_drain():
    """Split the tail Drain's sync waits (>1 rejected by this walrus)."""
    global _drain_patched
    if _drain_patched:
        return

    def _drain_and_barrier(self, tick_clock, wait_clock):
        nc = self.nc
        drain_inst = nc.sync.drain()
        wait_clock.add_sem_waits(
            drain_inst.ins, ScopedClock({None: tick_clock.global_clock})
        )
        si = drain_inst.ins.sync_info
        if si is not None and si.on_wait and len(si.on_wait) > 1:
            extra = list(si.on_wait[1:])
            del si.on_wait[1:]
            name2sem = {
                getattr(s, "name", None): s
                for s in self.sems.allocated().values()
            }
            for w in extra:
                nc.sync.wait_ge(name2sem[w.ant_name], w.wait_value)
        nc.all_engine_barrier()
        popped = nc._tile_sem_poison_stack.pop()
        assert popped is self._sem_poison
        nc.clear_and_free_semaphores(list(self.sems.allocated().values()))
        nc.all_engine_barrier()

    tile.TileContext._drain_and_barrier = _drain_and_barrier
    _drain_patched = True


def _split_excess_waits(nc, limit=1):
    """Move all but `limit` sync waits of every instruction onto nop
    instructions inserted immediately before it on the same engine
    queue (in-order execution makes this equivalent)."""
    fn = nc.m.functions[0]
    for bb in fn.blocks:
        offenders = []
        for idx, inst in enumerate(bb.instructions):
            si = getattr(inst, "sync_info", None)
            if si is not None and si.on_wait and len(si.on_wait) > limit:
                offenders.append((idx, inst))
        if not offenders:
            continue
        cur = nc.cur_bb.bb if hasattr(nc.cur_bb, "bb") else nc.cur_bb
        new_insts = []
        off_map = {id(inst): inst for _, inst in offenders}
        for inst in bb.instructions:
            si = getattr(inst, "sync_info", None)
            if id(inst) in off_map:
                extra = list(si.on_wait[:-limit])
                keep = list(si.on_wait[-limit:])
                del si.on_wait[: len(si.on_wait) - limit]
                assert list(si.on_wait) == keep
                for w in extra:
                    nop = nc.engines[inst.engine].nop(nofuse=True).ins
                    # nop() appended itself to the current bb; relocate it
                    popped = cur.instructions.pop()
                    assert popped is nop
                    nop.sync_info = mybir.SyncInfo(on_wait=[w], on_update=[])
                    new_insts.append(nop)
            new_insts.append(inst)
        bb.instructions[:] = new_insts


def max_waits(nc):
    worst = 0
    for bb in nc.m.functions[0].blocks:
        for inst in bb.instructions:
            si = getattr(inst, "sync_info", None)
            if si is not None and si.on_wait:
                worst = max(worst, len(si.on_wait))
    return worst


def build_nc(D, J, VS):
    """One core: logits_t[VS, J] = (hs[J, D] @ wt[D, VS]).T in bf16."""
    _patch_tile_drain()
    KC = D // P
    NVG = math.ceil(VS / VG_W)

    nc = bass.Bass()
    hst = nc.dram_tensor("hst", [P, KC, J], BF16, kind="ExternalInput")
    wt = nc.dram_tensor("wt", [D, VS], BF16, kind="ExternalInput")
    logits_t = nc.dram_tensor("logits_t", [VS, J], BF16, kind="ExternalOutput")
    wt_r = wt.rearrange("(k p) v -> p k v", p=P)

    with tile.TileContext(nc) as tc:
        with (
            tc.tile_pool(name="hs", bufs=1) as hs_pool,
            tc.tile_pool(name="w", bufs=3) as w_pool,
            tc.tile_pool(name="out", bufs=6) as out_pool,
            tc.tile_pool(name="ps", bufs=6, space=bass.MemorySpace.PSUM) as ps_pool,
        ):
            hst_sb = hs_pool.tile([P, KC, J], BF16)
            nc.gpsimd.dma_start(hst_sb[:], hst[:])

            for vg in range(NVG):
                vgw = min(VG_W, VS - vg * VG_W)
                w_sb = w_pool.tile([P, KC, VG_W], BF16)
                nc.sync.dma_start(
                    w_sb[:, :, :vgw], wt_r[:, :, vg * VG_W : vg * VG_W + vgw]
                )
                for sub in range(vgw // P):
                    ps = ps_pool.tile([P, 256], F32)
                    for k in range(KC):
                        nc.tensor.matmul(
                            ps[:, :J],
                            w_sb[:, k, sub * P : (sub + 1) * P],
                            hst_sb[:, k, :],
                            start=(k == 0),
                            stop=(k == KC - 1),
                        )
                    ot = out_pool.tile([P, J], BF16)
                    nc.vector.tensor_copy(ot[:], ps[:, :J])
                    row0 = vg * VG_W + sub * P
                    nc.scalar.dma_start(logits_t[row0 : row0 + P, :], ot[:])

    _split_excess_waits(nc, limit=1)
    return nc


def _job_indices(fill_tokens_num, num_generation_jobs):
    fill = np.asarray(fill_tokens_num, dtype=np.int64)
    fill_last = np.cumsum(fill) - 1
    total_fill = int(fill.sum())
    gen = total_fill + np.arange(int(num_generation_jobs), dtype=np.int64)
    return np.concatenate([fill_last, gen])


def kernel(hidden_states, embd_weight, fill_tokens_num, num_generation_jobs):
    hs = np.asarray(hidden_states, dtype=np.float32)
    W = np.asarray(embd_weight, dtype=np.float32)
    V, D = W.shape

    idx = _job_indices(fill_tokens_num, num_generation_jobs)
    J = idx.size

    hs_sel = hs[idx]  # [J, D] f32, kept for the exact rescore
    # [P, KC, J]: hst[p, k, j] = hs_sel[j, k*128 + p]
    hst_host = np.ascontiguousarray(
        hs_sel.T.reshape(D // P, P, J).transpose(1, 0, 2)
    ).astype(ml_dtypes.bfloat16)

    VS = math.ceil(V / (N_CORES * P)) * P  # per-core vocab shard width
    V_pad = VS * N_CORES
    Wb = W.astype(ml_dtypes.bfloat16)
    WT_pad = np.zeros((D, V_pad), dtype=ml_dtypes.bfloat16)
    WT_pad[:, :V] = Wb.T
    shards = [
        np.ascontiguousarray(WT_pad[:, i * VS : (i + 1) * VS])
        for i in range(N_CORES)
    ]

    nc = build_nc(D, J, VS)
    kernel.last_nc = nc
    kernel.last_in_maps = [{"hst": hst_host, "wt": shards[i]} for i in range(N_CORES)]
    res = run_bass_kernel_spmd(
        nc, kernel.last_in_maps, core_ids=list(range(N_CORES))
    )
    kernel.last_results = res

    # [J, V]: concat vocab shards (transposed on device), crop padding
    logits = np.concatenate(
        [res.results[i]["logits_t"].astype(np.float32) for i in range(N_CORES)],
        axis=0,
    ).T[:, :V]

    # Candidate columns within DELTA of each row's max, rescored exactly.
    m = logits.max(axis=1, keepdims=True)
    rows, cols = np.nonzero(logits >= m - DELTA)
    exact = np.einsum(
        "ij,ij->i", hs_sel[rows].astype(np.float64), W[cols].astype(np.float64)
    )
    ids = np.zeros(J, dtype=np.int64)
    best = np.full(J, -np.inf)
    for r, c, s in zip(rows, cols, exact):
        if s > best[r]:
            best[r] = s
            ids[r] = c
    return ids.astype(np.int32)


# revision 5
# speedup vs baseline: 134.1330x; 134.1330x over previous
"""GreedySampler kernel for 8 Trainium2 NeuronCores.

The reference gathers 200 "last token" rows of hidden_states (8
prefill ends + 192 decode slots), computes logits against the
50257x4096 embedding matrix, and takes the argmax over vocab (softmax
and log are monotonic, so argmax(logits) is the answer). The dominant
cost is streaming the 823MB embedding matrix: memory-bound.

Plan:
  * Host: compute gather indices from fill_tokens_num /
    num_generation_jobs, gather the 200 rows, transpose to the PE's
    [K, M] layout. Scale embd_weight by 64 (centers its sigma=0.02
    values in fp8-e4m3's normal range), cast both operands to e4m3,
    transpose W to [d, vocab], pad vocab to 51200, shard over vocab
    into 8 slices of 6400 columns (tensor-parallel over vocab).
  * Device (SPMD, 8 cores): stream the 26MB W-shard once (~73us at
    358GB/s/core, the roofline). W chunks are the stationary matmul
    operand, the 200 job rows the moving operand, with fp8 DoubleRow
    packing K=256 per pass: the PE does 200*4096*6400 MACs in ~160k
    cycles (~67us), under the DMA floor. Accumulation is fp32 in PSUM;
    logits.T[vocab_shard, 200] goes out as bf16.
  * Host: per-row global max over the gathered approximate logits;
    every column within DELTA of the max (fp8 logit error measured at
    <=0.28 in unscaled units; DELTA=2.0 is a ~7x margin on the max
    observed error, ~30 sigma) is rescored exactly in float64 against
    the original fp32 weights. The argmax of exact scores equals the
    fp32 reference argmax — quantization only shortlists candidates,
    it never decides the winner.

Notes:
  * This walrus build rejects instructions carrying more than one sync
    wait, so after Tile scheduling we split excess waits onto nop
    instructions inserted just before the offender on the same engine
    queue (in-order execution keeps the semantics identical).
  * DoubleRow AP contract: lhsT [128, 2, M] (free = 2M), rhs
    [128, 2, N] (free = 2N), out [M, N]; both operands here use
    d = kk*256 + t*128 + p so the packing is consistent.
"""

import math

import numpy as np
import ml_dtypes

import concourse.bass as bass
import concourse.mybir as mybir
import concourse.tile as tile
from concourse.vector_clock import ScopedClock
from concourse.bass_utils import run_bass_kernel_spmd

P = 128
N_CORES = 8
VG_W = 512  # W-tile width in vocab (4 stationary tiles of 128)
W_SCALE = 64.0
DELTA = 2.0 * W_SCALE  # candidate margin in scaled-logit units

FP8 = mybir.dt.float8e4
F32 = mybir.dt.float32
BF16 = mybir.dt.bfloat16

_drain_patched = False


def _patch_tile_drain():
    """Split the tail Drain's sync waits (>1 rejected by this walrus)."""
    global _drain_patched
    if _drain_patched:
        return

    def _drain_and_barrier(self, tick_clock, wait_clock):
        nc = self.nc
        drain_inst = nc.sync.drain()
        wait_clock.add_sem_waits(
            drain_inst.ins, ScopedClock({None: tick_clock.global_clock})
        )
        si = drain_inst.ins.sync_info
        if si is not None and si.on_wait and len(si.on_wait) > 1:
            extra = list(si.on_wait[1:])
            del si.on_wait[1:]
            name2sem = {
                getattr(s, "name", None): s
                for s in self.sems.allocated().values()
            }
            for w in extra:
                nc.sync.wait_ge(name2sem[w.ant_name], w.wait_value)
        nc.all_engine_barrier()
        popped = nc._tile_sem_poison_stack.pop()
        assert popped is self._sem_poison
        nc.clear_and_free_semaphores(list(self.sems.allocated().values()))
        nc.all_engine_barrier()

    tile.TileContext._drain_and_barrier = _drain_and_barrier
    _drain_patched = True


def _split_excess_waits(nc, limit=1):
    """Move all but `limit` sync waits of every instruction onto nops
    inserted immediately before it on the same engine queue."""
    fn = nc.m.functions[0]
    for bb in fn.blocks:
        if not any(
            getattr(i, "sync_info", None) is not None
            and i.sync_info.on_wait
            and len(i.sync_info.on_wait) > limit
            for i in bb.instructions
        ):
            continue
        cur = nc.cur_bb.bb if hasattr(nc.cur_bb, "bb") else nc.cur_bb
        new_insts = []
        for inst in bb.instructions:
            si = getattr(inst, "sync_info", None)
            if si is not None and si.on_wait and len(si.on_wait) > limit:
                extra = list(si.on_wait[:-limit])
                del si.on_wait[: len(si.on_wait) - limit]
                for w in extra:
                    nop = nc.engines[inst.engine].nop(nofuse=True).ins
                    popped = cur.instructions.pop()  # nop() self-appended
                    assert popped is nop
                    nop.sync_info = mybir.SyncInfo(on_wait=[w], on_update=[])
                    new_insts.append(nop)
            new_insts.append(inst)
        bb.instructions[:] = new_insts


def max_waits(nc):
    worst = 0
    for bb in nc.m.functions[0].blocks:
        for inst in bb.instructions:
            si = getattr(inst, "sync_info", None)
            if si is not None and si.on_wait:
                worst = max(worst, len(si.on_wait))
    return worst


def build_nc(D, J, VS):
    """One core: logits_t[VS, J] = (hs[J, D] @ wt[D, VS]).T, fp8 in,
    bf16 out, fp32 accumulation."""
    _patch_tile_drain()
    KK = D // (2 * P)  # 16 DoubleRow K-chunks of 256
    NVG = math.ceil(VS / VG_W)

    nc = bass.Bass()
    hst = nc.dram_tensor("hst", [P, KK, 2, J], FP8, kind="ExternalInput")
    wt = nc.dram_tensor("wt", [D, VS], FP8, kind="ExternalInput")
    logits_t = nc.dram_tensor("logits_t", [VS, J], BF16, kind="ExternalOutput")
    wt_r = wt.rearrange("(kk t p) v -> p kk t v", p=P, t=2)

    with tile.TileContext(nc) as tc:
        with (
            tc.tile_pool(name="hs", bufs=1) as hs_pool,
            tc.tile_pool(name="w", bufs=4) as w_pool,
            tc.tile_pool(name="out", bufs=6) as out_pool,
            tc.tile_pool(name="ps", bufs=6, space=bass.MemorySpace.PSUM) as ps_pool,
        ):
            hst_sb = hs_pool.tile([P, KK, 2, J], FP8)
            nc.gpsimd.dma_start(hst_sb[:], hst[:])

            for vg in range(NVG):
                vgw = min(VG_W, VS - vg * VG_W)
                nsub = vgw // P
                w_sb = w_pool.tile([P, KK, 2, VG_W], FP8, name="w_sb")
                nc.sync.dma_start(
                    w_sb[:, :, :, :vgw],
                    wt_r[:, :, :, vg * VG_W : vg * VG_W + vgw],
                )
                ot = out_pool.tile([P, 4, J], BF16, name="ot")
                for sub in range(nsub):
                    ps = ps_pool.tile([P, 256], F32, name="ps")
                    for kk in range(KK):
                        nc.tensor.matmul(
                            ps[:, :J],
                            w_sb[:, kk, :, sub * P : (sub + 1) * P],
                            hst_sb[:, kk, :, :],
                            start=(kk == 0),
                            stop=(kk == KK - 1),
                            perf_mode=mybir.MatmulPerfMode.DoubleRow,
                        )
                    nc.vector.tensor_copy(ot[:, sub, :], ps[:, :J])
                dst = logits_t[vg * VG_W : vg * VG_W + vgw, :].rearrange(
                    "(s p) j -> p s j", p=P
                )
                nc.scalar.dma_start(dst, ot[:, :nsub, :])

    _split_excess_waits(nc, limit=1)
    return nc


def _job_indices(fill_tokens_num, num_generation_jobs):
    fill = np.asarray(fill_tokens_num, dtype=np.int64)
    fill_last = np.cumsum(fill) - 1
    total_fill = int(fill.sum())
    gen = total_fill + np.arange(int(num_generation_jobs), dtype=np.int64)
    return np.concatenate([fill_last, gen])


def kernel(hidden_states, embd_weight, fill_tokens_num, num_generation_jobs):
    hs = np.asarray(hidden_states, dtype=np.float32)
    W = np.asarray(embd_weight, dtype=np.float32)
    V, D = W.shape

    idx = _job_indices(fill_tokens_num, num_generation_jobs)
    J = idx.size

    hs_sel = hs[idx]  # [J, D] f32, kept for the exact rescore
    # [P, KK, 2, J]: hst[p, kk, t, j] = hs_sel[j, kk*256 + t*128 + p]
    hst_host = np.ascontiguousarray(
        hs_sel.T.reshape(D // 256, 2, P, J).transpose(2, 0, 1, 3)
    ).astype(ml_dtypes.float8_e4m3)

    VS = math.ceil(V / (N_CORES * P)) * P  # per-core vocab shard width
    V_pad = VS * N_CORES
    Wq = (W * W_SCALE).astype(ml_dtypes.float8_e4m3)
    WT_pad = np.zeros((D, V_pad), dtype=ml_dtypes.float8_e4m3)
    WT_pad[:, :V] = Wq.T
    shards = [
        np.ascontiguousarray(WT_pad[:, i * VS : (i + 1) * VS])
        for i in range(N_CORES)
    ]

    nc = build_nc(D, J, VS)
    kernel.last_nc = nc
    kernel.last_in_maps = [
        {"hst": hst_host, "wt": shards[i]} for i in range(N_CORES)
    ]
    res = run_bass_kernel_spmd(
        nc, kernel.last_in_maps, core_ids=list(range(N_CORES))
    )
    kernel.last_results = res

    # [J, V_pad] -> crop pad; values are scaled by W_SCALE (irrelevant
    # for ranking, DELTA is in the same scaled units)
    logits = np.concatenate(
        [res.results[i]["logits_t"].astype(np.float32) for i in range(N_CORES)],
        axis=0,
    ).T[:, :V]

    # Columns within DELTA of each row's max, rescored exactly in f64.
    m = logits.max(axis=1, keepdims=True)
    rows, cols = np.nonzero(logits >= m - DELTA)
    exact = np.einsum(
        "ij,ij->i", hs_sel[rows].astype(np.float64), W[cols].astype(np.float64)
    )
    ids = np.zeros(J, dtype=np.int64)
    best = np.full(J, -np.inf)
    for r, c, s in zip(rows, cols, exact):
        if s > best[r]:
            best[r] = s
            ids[r] = c
    return ids.astype(np.int32)
